# revision 1
# baseline (speedup 1.0000x reference)
"""BatchAllTripletLoss on 8 Trainium2 NeuronCores via Bass/Tile.

Math: for anchors i, positives j (same label, j!=i), negatives k (diff label):
  total        = sum_{i,j,k} relu(d_ij - d_ik + margin)
  num_non_easy = #{(i,j,k): d_ik < d_ij + margin}
  loss         = total / num_non_easy ; frac = num_non_easy / num_valid

Sharding: anchors i split 80 per core; each core computes its [80, 640]
distance-row block with PE matmuls.

O(n^3) strategy (per anchor a):
  - masked row v'_k = d_ak + BIG*(same label), bf16, staged to DRAM and
    DMA-broadcast to [128, 640] (stride-0 partition source).
  - masked thresholds t'_j = (d_aj + margin) * positive_mask (0 when not a
    positive pair), kept f32 per-partition and split hi/lo into bf16.
  - DVE builds the 0/1 matrix M[j, k] = (v'_k < t'_j) in ONE bf16
    tensor_scalar (is_lt, no accum -> 4x mode, ~300ns per [128,640] tile).
    Every 4th anchor instead uses ACT Sign (M = (sign+1)/2, corrected on the
    host with threshold/distance row sums) to use the idle Scalar engine.
  - PE reduces M with lhsT = [t'_hi | t'_lo | 1 | 0] (bf16):
       psum row base+0/1: sum_j t'_j * M[j,k]  (hi/lo parts)
       psum row base+2:   q[k] = sum_j M[j,k]
    accumulated over the 5 j-tiles; 3 anchors per psum tile (bases 0/32/64).
  - ACT free-sums psum rows (Identity + accum); DVE does the fused
    (q * dist) reduce for the  sum_k d_ak * q_ak  term.
  total = sum(t'*M) - sum(d*q);  count = sum(q).  Host combines in f64.
num_valid is pure label counting (host, exact).
"""

import numpy as np

N = 640
D = 128
NCORES = 8
NLOC = N // NCORES            # 80 anchors per core
NCT = N // 128                # 5 j-tiles of 128
NGRP = (NLOC + 2) // 3        # 27 psum groups, 3 anchors each (last has 2)
MARGIN = 1.9
BIG = 1.0e9

_CACHE = {}


def _build_program():
    import concourse.bass as bass
    import concourse.bacc as bacc
    import concourse.mybir as mybir
    import concourse.tile as tile
    from concourse.masks import make_identity

    f32 = mybir.dt.float32
    bf16 = mybir.dt.bfloat16
    Alu = mybir.AluOpType
    Act = mybir.ActivationFunctionType

    nc = bacc.Bacc("TRN2", target_bir_lowering=False, debug=False,
                   num_devices=NCORES)

    efT = nc.declare_dram_parameter("efT", [D, N], f32, isOutput=False)
    elocT = nc.declare_dram_parameter("elocT", [D, NLOC], f32, isOutput=False)
    labrow = nc.declare_dram_parameter("labrow", [1, N], f32, isOutput=False)
    labT = nc.declare_dram_parameter("labT", [128, NCT], f32, isOutput=False)
    llocrow = nc.declare_dram_parameter("llocrow", [1, NLOC], f32, isOutput=False)
    llocT = nc.declare_dram_parameter("llocT", [NLOC, 1], f32, isOutput=False)
    eye = nc.declare_dram_parameter("eye", [128, NCT * NLOC], f32, isOutput=False)
    # out: [128, 2*NGRP(wsums) + 2*NGRP(p2) + 1 (dist row sums)] = [128, 109]
    out_d = nc.declare_dram_parameter("out", [128, 4 * NGRP + 1], f32,
                                      isOutput=True)
    # out2: [1, NLOC] Tsum_a = sum_j t'_aj (for ACT-sign anchors' corrections)
    out2_d = nc.declare_dram_parameter("out2", [1, NLOC], f32, isOutput=True)

    from contextlib import ExitStack
    with tile.TileContext(nc) as tc:
        with (
            tc.tile_pool(name="singles", bufs=1) as sg,
            tc.tile_pool(name="vbp", bufs=6) as vbp,
            tc.tile_pool(name="mtp", bufs=10) as mtp,
            tc.tile_pool(name="dpp", bufs=2) as dpp,
            tc.tile_pool(name="drs", bufs=3) as drs,
            tc.tile_pool(name="dram", bufs=1, space="DRAM") as dram,
        ):
            pro_stack = ExitStack()
            ps_mm = pro_stack.enter_context(
                tc.tile_pool(name="ps_mm", bufs=1, space="PSUM"))
            ps_tr = pro_stack.enter_context(
                tc.tile_pool(name="ps_tr", bufs=1, space="PSUM"))
            # ---- load inputs ----
            EF = sg.tile([D, N], f32)
            nc.gpsimd.dma_start(out=EF[:], in_=efT[:])
            EL = sg.tile([D, NLOC], f32)
            nc.gpsimd.dma_start(out=EL[:], in_=elocT[:])
            LR = sg.tile([1, N], f32)
            nc.gpsimd.dma_start(out=LR[:], in_=labrow[:])
            LT = sg.tile([128, NCT], f32)
            nc.gpsimd.dma_start(out=LT[:], in_=labT[:])
            LLR = sg.tile([1, NLOC], f32)
            nc.gpsimd.dma_start(out=LLR[:], in_=llocrow[:])
            LLT = sg.tile([NLOC, 1], f32)
            nc.gpsimd.dma_start(out=LLT[:], in_=llocT[:])
            EYE = sg.tile([128, NCT * NLOC], f32)
            nc.gpsimd.dma_start(out=EYE[:], in_=eye[:])

            ident = sg.tile([128, 128], f32)
            make_identity(nc, ident[:])
            ones = sg.tile([128, 1], f32)
            nc.vector.memset(ones[:], 1.0)

            # ---- pairwise distance rows for local anchors ----
            Esq = sg.tile([D, N], f32)
            nc.vector.tensor_mul(Esq[:], EF[:], EF[:])
            ELsq = sg.tile([D, NLOC], f32)
            nc.vector.tensor_mul(ELsq[:], EL[:], EL[:])

            sqf_ps = ps_mm.tile([1, N], f32, tag="pro", name="sqf")
            nc.tensor.matmul(sqf_ps[:, 0:512], ones[:], Esq[:, 0:512])
            nc.tensor.matmul(sqf_ps[:, 512:N], ones[:], Esq[:, 512:N])
            SQF = sg.tile([1, N], f32)
            nc.vector.tensor_copy(SQF[:], sqf_ps[:])

            sql_ps = ps_mm.tile([NLOC, 1], f32, tag="pro", name="sql")
            nc.tensor.matmul(sql_ps[:], ELsq[:], ones[:])
            SQL = sg.tile([NLOC, 1], f32)
            nc.vector.tensor_copy(SQL[:], sql_ps[:])

            dot_ps = ps_mm.tile([NLOC, N], f32, tag="pro", name="dot")
            nc.tensor.matmul(dot_ps[:, 0:512], EL[:], EF[:, 0:512])
            nc.tensor.matmul(dot_ps[:, 512:N], EL[:], EF[:, 512:N])

            A = sg.tile([NLOC, N], f32)
            nc.vector.tensor_scalar(out=A[:], in0=dot_ps[:], scalar1=-2.0,
                                    scalar2=SQL[:], op0=Alu.mult, op1=Alu.add)
            sqf_d = dram.tile([1, N], f32)
            nc.sync.dma_start(out=sqf_d[:], in_=SQF[:])
            SQB = sg.tile([128, N], f32)
            nc.sync.dma_start(out=SQB[0:NLOC, :],
                              in_=sqf_d[:].to_broadcast([NLOC, N]))
            PRE = sg.tile([NLOC, N], f32)
            nc.vector.tensor_add(PRE[:], A[:], SQB[0:NLOC, :])
            nc.vector.tensor_scalar(out=PRE[:], in0=PRE[:], scalar1=0.0,
                                    scalar2=None, op0=Alu.max)
            DIST = sg.tile([NLOC, N], f32)
            nc.scalar.activation(out=DIST[:], in_=PRE[:], func=Act.Sqrt)

            # masked v' row, bf16
            LBC = sg.tile([128, N], f32)
            nc.sync.dma_start(out=LBC[0:NLOC, :],
                              in_=labrow[:].to_broadcast([NLOC, N]))
            EQB = sg.tile([NLOC, N], f32)
            nc.vector.tensor_scalar(out=EQB[:], in0=LBC[0:NLOC, :], scalar1=LLT[:],
                                    scalar2=BIG, op0=Alu.is_equal, op1=Alu.mult)
            VM = sg.tile([NLOC, N], f32)
            nc.vector.tensor_add(VM[:], DIST[:], EQB[:])
            VMB = sg.tile([NLOC, N], bf16)
            nc.vector.tensor_copy(VMB[:], VM[:])
            vmd = dram.tile([NLOC, N], bf16)
            nc.sync.dma_start(out=vmd[:], in_=VMB[:])

            # positive mask transposed: (lab_j == lab_a) - eye
            LLB = sg.tile([128, NLOC], f32)
            nc.sync.dma_start(out=LLB[:],
                              in_=llocrow[:].to_broadcast([128, NLOC]))
            posT = []
            for c in range(NCT):
                p = sg.tile([128, NLOC], f32, tag=f"posT{c}", name=f"posT{c}")
                nc.vector.tensor_scalar(out=p[:], in0=LLB[:], scalar1=LT[:, c:c + 1],
                                        scalar2=None, op0=Alu.is_equal)
                nc.vector.tensor_sub(p[:], p[:], EYE[:, c * NLOC:(c + 1) * NLOC])
                posT.append(p)

            # thresholds: tp[c][p, a] = (dist[a, c*128+p] + margin) * posT
            # plus bf16 hi/lo split packed into lhsT tiles [128, NLOC, 4]
            tp = []
            lhsb = []
            for c in range(NCT):
                tr_ps = ps_tr.tile([128, NLOC], f32, tag="tr")
                nc.tensor.transpose(tr_ps[:], DIST[:, c * 128:(c + 1) * 128],
                                    ident[0:NLOC, 0:NLOC])
                t = sg.tile([128, NLOC], f32, tag=f"tp{c}", name=f"tp{c}")
                nc.vector.tensor_scalar_add(out=t[:], in0=tr_ps[:], scalar1=MARGIN)
                nc.vector.tensor_mul(t[:], t[:], posT[c][:])
                tp.append(t)

                L = sg.tile([128, NLOC, 4], bf16, tag=f"lhsb{c}", name=f"lhsb{c}")
                nc.vector.memset(L[:], 0.0)
                nc.vector.tensor_copy(L[:, :, 0], t[:])            # t_hi (bf16)
                thf = sg.tile([128, NLOC], f32, tag="thf", name="thf")
                nc.vector.tensor_copy(thf[:], L[:, :, 0])          # back to f32
                nc.vector.tensor_sub(thf[:], t[:], thf[:])         # t_lo
                nc.vector.tensor_copy(L[:, :, 1], thf[:])
                nc.vector.memset(L[:, :, 2], 1.0)
                lhsb.append(L)

            # dist row sums (for sign-anchor corrections)
            DSC = sg.tile([NLOC, N], f32)
            DSUM = sg.tile([NLOC, 1], f32)
            nc.scalar.activation(out=DSC[:], in_=DIST[:], func=Act.Identity,
                                 bias=0.0, scale=1.0, accum_out=DSUM[:])

            # Tsum_a = sum_j t'_aj : ones^T @ tp[c], accumulated over c
            ts_ps = ps_tr.tile([1, NLOC], f32, tag="tr", name="ts_ps")
            for c in range(NCT):
                nc.tensor.matmul(ts_ps[:], ones[:], tp[c][:],
                                 start=(c == 0), stop=(c == NCT - 1))
            TSROW = sg.tile([1, NLOC], f32)
            nc.vector.tensor_copy(TSROW[:], ts_ps[:])
            nc.sync.dma_start(out=out2_d[:], in_=TSROW[:])

            pro_stack.close()
            wq_stack = ExitStack()
            ps_wq1 = wq_stack.enter_context(
                tc.tile_pool(name="ps_wq1", bufs=3, space="PSUM"))
            ps_wq2 = wq_stack.enter_context(
                tc.tile_pool(name="ps_wq2", bufs=3, space="PSUM"))

            # ---- main loop ----
            vb2_cache = {}
            dr_tiles = []   # (DR accum tile [128, 2], P2 accum tile [128, 2])
            for g in range(NGRP):
                na = min(3, NLOC - 3 * g)
                wq1 = ps_wq1.tile([128, 512], f32, tag="wq1", name="wq1")
                wq2 = ps_wq2.tile([128, 128], f32, tag="wq2", name="wq2")
                dp = dpp.tile([128, N], f32, tag="dp", name="dp")
                for m in range(na):
                    a = 3 * g + m
                    base = 32 * m
                    if a % 2 == 0:
                        nanch = min(2, NLOC - a)
                        vb2 = vbp.tile([128, 2, N], bf16, tag="vb", name="vb")
                        sl = vmd[a:a + nanch, :]
                        bsrc = bass.AP(tensor=sl.tensor, offset=sl.offset,
                                       ap=[[0, 128]] + [list(p) for p in sl.ap])
                        nc.sync.dma_start(out=vb2[:, 0:nanch, :], in_=bsrc)
                        vb2_cache[0] = vb2
                    vb = vb2_cache[0][:, a % 2, :]
                    nc.sync.dma_start(out=dp[base + 2:base + 3, :],
                                      in_=DIST[a:a + 1, :])
                    on_act = (a % 4 == 1)
                    for c in range(NCT):
                        mt = mtp.tile([128, N], bf16, tag="mt", name="mt")
                        if on_act:
                            nc.scalar.activation(out=mt[:], in_=vb[:],
                                                 func=Act.Sign,
                                                 bias=tp[c][:, a:a + 1],
                                                 scale=-1.0)
                        else:
                            nc.vector.tensor_scalar(out=mt[:], in0=vb[:],
                                                    scalar1=tp[c][:, a:a + 1],
                                                    scalar2=None, op0=Alu.is_lt)
                        nc.tensor.matmul(wq1[base:base + 4, :],
                                         lhsb[c][:, a], mt[:, 0:512],
                                         start=(c == 0), stop=(c == NCT - 1))
                        nc.tensor.matmul(wq2[base:base + 4, :],
                                         lhsb[c][:, a], mt[:, 512:N],
                                         start=(c == 0), stop=(c == NCT - 1))
                # drain group: ACT free-sums all psum rows; DVE fused q*dist
                DR = drs.tile([128, 2], f32, tag="dr", name="dr")
                P2 = drs.tile([128, 2], f32, tag="p2", name="p2")
                sa1 = drs.tile([128, 512], f32, tag="sa1", name="sa1")
                sa2 = drs.tile([128, 128], f32, tag="sa2", name="sa2")
                sb1 = drs.tile([128, 512], f32, tag="sb1", name="sb1")
                sb2 = drs.tile([128, 128], f32, tag="sb2", name="sb2")
                nc.scalar.activation(out=sa1[:], in_=wq1[:], func=Act.Identity,
                                     bias=0.0, scale=1.0, accum_out=DR[:, 0:1])
                nc.scalar.activation(out=sa2[:], in_=wq2[:], func=Act.Identity,
                                     bias=0.0, scale=1.0, accum_out=DR[:, 1:2])
                nc.vector.scalar_tensor_tensor(out=sb1[:], in0=wq1[:],
                                               scalar=1.0, in1=dp[:, 0:512],
                                               op0=Alu.mult, op1=Alu.mult,
                                               accum_out=P2[:, 0:1])
                nc.vector.scalar_tensor_tensor(out=sb2[:], in0=wq2[:],
                                               scalar=1.0, in1=dp[:, 512:N],
                                               op0=Alu.mult, op1=Alu.mult,
                                               accum_out=P2[:, 1:2])
                dr_tiles.append((DR, P2))

            # ---- stage outputs ----
            OUTS = sg.tile([128, 4 * NGRP + 1], f32)
            nc.vector.tensor_copy(OUTS[0:NLOC, 4 * NGRP:4 * NGRP + 1], DSUM[:])
            for g, (DR, P2) in enumerate(dr_tiles):
                nc.vector.tensor_copy(OUTS[:, 2 * g:2 * g + 2], DR[:])
                nc.vector.tensor_copy(OUTS[:, 2 * NGRP + 2 * g:2 * NGRP + 2 * g + 2],
                                      P2[:])
            nc.gpsimd.dma_start(out=out_d[:], in_=OUTS[:])
            wq_stack.close()

    nc.compile()
    return nc


def _get_program():
    if "nc" not in _CACHE:
        _CACHE["nc"] = _build_program()
    return _CACHE["nc"]


def _make_inputs(embeddings: np.ndarray, labels: np.ndarray):
    e = np.ascontiguousarray(embeddings.reshape(N, D).astype(np.float32))
    lab = labels.reshape(N).astype(np.float32)
    efT = np.ascontiguousarray(e.T)                       # [D, N]
    labrow = lab.reshape(1, N)
    labT = np.ascontiguousarray(lab.reshape(NCT, 128).T)  # [128, NCT]

    in_maps = []
    for r in range(NCORES):
        g0 = r * NLOC
        eye = np.zeros((128, NCT * NLOC), np.float32)
        for a in range(NLOC):
            j = g0 + a
            eye[j % 128, (j // 128) * NLOC + a] = 1.0
        in_maps.append({
            "efT": efT,
            "elocT": np.ascontiguousarray(efT[:, g0:g0 + NLOC]),
            "labrow": labrow,
            "labT": labT,
            "llocrow": np.ascontiguousarray(lab[g0:g0 + NLOC].reshape(1, NLOC)),
            "llocT": np.ascontiguousarray(lab[g0:g0 + NLOC].reshape(NLOC, 1)),
            "eye": eye,
        })
    return in_maps


def run_on_device(embeddings: np.ndarray, labels: np.ndarray, **run_kwargs):
    from concourse.bass_utils import run_bass_kernel_spmd
    nc = _get_program()
    in_maps = _make_inputs(embeddings, labels)
    res = run_bass_kernel_spmd(nc, in_maps, core_ids=list(range(NCORES)),
                               **run_kwargs)
    total = 0.0
    count = 0.0
    for r in range(NCORES):
        o = res.results[r]["out"].astype(np.float64)
        tsum = res.results[r]["out2"].astype(np.float64).reshape(-1)
        dsum = o[0:NLOC, 4 * NGRP]
        for g in range(NGRP):
            na = min(3, NLOC - 3 * g)
            for m in range(na):
                a = 3 * g + m
                base = 32 * m
                w = q = p2 = 0.0
                for ch in range(2):
                    w += o[base + 0, 2 * g + ch] + o[base + 1, 2 * g + ch]
                    q += o[base + 2, 2 * g + ch]
                    p2 += o[base + 2, 2 * NGRP + 2 * g + ch]
                if a % 4 == 1:   # sign anchor
                    w = 0.5 * w + 0.5 * N * tsum[a]
                    q = 0.5 * q + 0.5 * N * N
                    p2 = 0.5 * p2 + 0.5 * N * dsum[a]
                total += w - p2
                count += q
    return total, count, res


def kernel(embeddings: np.ndarray, labels: np.ndarray):
    embeddings = np.asarray(embeddings)
    labels = np.asarray(labels)
    total, count, _ = run_on_device(embeddings, labels)

    lab = np.asarray(labels).reshape(-1)
    cnt = np.bincount(lab.astype(np.int64), minlength=1)
    per = cnt[lab.astype(np.int64)]
    num_valid = int(((per - 1) * (N - per)).sum())

    nv = np.float32(num_valid)
    ne = np.float32(count)
    tot = np.float32(total)
    if ne > 0:
        loss = np.float32(tot / np.maximum(ne, np.float32(1.0)))
    else:
        loss = np.float32(0.0)
    frac = np.float32(ne / (nv + np.float32(1e-16)))
    return (np.array(loss, np.float32), np.array(nv, np.float32),
            np.array(ne, np.float32), np.array(frac, np.float32))



# revision 7
# speedup vs baseline: 2.0965x; 2.0965x over previous
"""BatchAllTripletLoss on 8 Trainium2 NeuronCores via Bass/Tile.

Math: for anchors i, positives j (same label, j!=i), negatives k (diff label):
  total        = sum_{i,j,k} relu(d_ij - d_ik + margin)
  num_non_easy = #{(i,j,k): d_ik < d_ij + margin}
  loss         = total / num_non_easy ; frac = num_non_easy / num_valid

Key idea: samples are SORTED BY LABEL on the host, so each anchor's
positives j live in one contiguous window of at most 128 sorted positions.
The O(n^3) triplet work then needs only ONE [128, 640] compare tile per
anchor (j = its class window, k = all samples) instead of five:
  - masked row v'_k = d_ak + BIG*(same label), bf16, staged to DRAM and
    DMA-broadcast to [128, 640] (stride-0 partition source).
  - window thresholds t'_p = (d_{a, w_a+p} + margin) * posmask, gathered
    from the distance rows with ONE indirect DMA (per-anchor element
    offsets w_a are host data -> program is label-independent), f32,
    split hi/lo into bf16 lhsT columns [t_hi | t_lo | 1].
  - compare M[p, k] = (v'_k < t'_p): DVE is_lt bf16 (~300ns) for 3 of 4
    anchors, ACT Sign (host-corrected) for the rest.
  - PE reduces M: psum rows 32s..32s+2 (4 anchors per [128, 640] psum
    tile via explicit tile_position), giving sum_p t'_p M, and q_k.
  - one drain per psum tile: ACT Identity (+accum) -> W and count; DVE
    scalar_tensor_tensor (*dist rows, +accum) -> sum_k d_ak q_ak.
  total = sum(t'*M) - sum(d*q);  count = sum(q).  Host combines in f64.
num_valid is pure label counting (host, exact).
"""

import numpy as np

N = 640
D = 128
NCORES = 8
NLOC = N // NCORES            # 80 anchors per core
NT = NLOC // 4                # 20 psum tiles, 4 anchors each
MARGIN = 1.9
BIG = 1.0e9
WMAX = 128                    # class-window width (max class size)

_CACHE = {}


def _is_act(la):
    return la % 4 == 1


def _build_program():
    import concourse.bass as bass
    import concourse.bacc as bacc
    import concourse.mybir as mybir
    import concourse.tile as tile
    from concourse.masks import make_identity

    f32 = mybir.dt.float32
    bf16 = mybir.dt.bfloat16
    i32 = mybir.dt.int32
    Alu = mybir.AluOpType
    Act = mybir.ActivationFunctionType

    nc = bacc.Bacc("TRN2", target_bir_lowering=False, debug=False,
                   num_devices=NCORES)

    efT = nc.declare_dram_parameter("efT", [D, N], f32, isOutput=False)
    elocT = nc.declare_dram_parameter("elocT", [D, NLOC], f32, isOutput=False)
    labrow = nc.declare_dram_parameter("labrow", [1, N], f32, isOutput=False)
    llocT = nc.declare_dram_parameter("llocT", [NLOC, 1], f32, isOutput=False)
    woff = nc.declare_dram_parameter("woff", [NLOC, 1], i32, isOutput=False)
    posw = nc.declare_dram_parameter("posw", [WMAX, NLOC], f32, isOutput=False)
    # out cols: 0:NT per-psum-tile row sums (W rows + q rows),
    #           NT:2NT the d*q sums, 2NT dist row sums
    OUTC = 2 * NT + 1
    out_d = nc.declare_dram_parameter("out", [128, OUTC], f32, isOutput=True)
    out2_d = nc.declare_dram_parameter("out2", [1, NLOC], f32, isOutput=True)

    from contextlib import ExitStack
    with tile.TileContext(nc) as tc:
        with (
            tc.tile_pool(name="singles", bufs=1) as sg,
            tc.tile_pool(name="vbp", bufs=8) as vbp,
            tc.tile_pool(name="mtp", bufs=10) as mtp,
            tc.tile_pool(name="drs", bufs=2) as drs,
            tc.tile_pool(name="dram", bufs=1, space="DRAM") as dram,
        ):
            pro_stack = ExitStack()
            ps_mm = pro_stack.enter_context(
                tc.tile_pool(name="ps_mm", bufs=1, space="PSUM"))
            ps_tr = pro_stack.enter_context(
                tc.tile_pool(name="ps_tr", bufs=1, space="PSUM"))
            # ---- load inputs ----
            EF = sg.tile([D, N], f32)
            nc.gpsimd.dma_start(out=EF[:], in_=efT[:])
            EL = sg.tile([D, NLOC], f32)
            nc.gpsimd.dma_start(out=EL[:], in_=elocT[:])
            LR = sg.tile([1, N], f32)
            nc.gpsimd.dma_start(out=LR[:], in_=labrow[:])
            LLT = sg.tile([NLOC, 1], f32)
            nc.gpsimd.dma_start(out=LLT[:], in_=llocT[:])
            WOFF = sg.tile([NLOC, 1], i32)
            nc.gpsimd.dma_start(out=WOFF[:], in_=woff[:])
            POSW = sg.tile([WMAX, NLOC], f32)
            nc.gpsimd.dma_start(out=POSW[:], in_=posw[:])

            ident = sg.tile([128, 128], f32)
            make_identity(nc, ident[:])
            ones = sg.tile([128, 1], f32)
            nc.vector.memset(ones[:], 1.0)

            # ---- pairwise distance rows for local anchors ----
            # dot products in bf16 (2x PE rate); squared norms in f32
            EFB = sg.tile([D, N], bf16)
            nc.vector.tensor_copy(EFB[:], EF[:])
            ELB = sg.tile([D, NLOC], bf16)
            nc.vector.tensor_copy(ELB[:], EL[:])
            Esq = sg.tile([D, N], f32)
            nc.vector.tensor_mul(Esq[:], EF[:], EF[:])
            ELsq = sg.tile([D, NLOC], f32)
            nc.vector.tensor_mul(ELsq[:], EL[:], EL[:])

            sqf_ps = ps_mm.tile([1, N], f32, tag="pro", name="sqf")
            nc.tensor.matmul(sqf_ps[:, 0:512], ones[:], Esq[:, 0:512])
            nc.tensor.matmul(sqf_ps[:, 512:N], ones[:], Esq[:, 512:N])
            SQF = sg.tile([1, N], f32)
            nc.vector.tensor_copy(SQF[:], sqf_ps[:])

            sql_ps = ps_mm.tile([NLOC, 1], f32, tag="pro", name="sql")
            nc.tensor.matmul(sql_ps[:], ELsq[:], ones[:])
            SQL = sg.tile([NLOC, 1], f32)
            nc.vector.tensor_copy(SQL[:], sql_ps[:])

            dot_ps = ps_mm.tile([NLOC, N], f32, tag="pro", name="dot")
            nc.tensor.matmul(dot_ps[:, 0:512], ELB[:], EFB[:, 0:512])
            nc.tensor.matmul(dot_ps[:, 512:N], ELB[:], EFB[:, 512:N])

            A = sg.tile([NLOC, N], f32)
            nc.vector.tensor_scalar(out=A[:], in0=dot_ps[:], scalar1=-2.0,
                                    scalar2=SQL[:], op0=Alu.mult, op1=Alu.add)
            sqf_d = dram.tile([1, N], f32)
            nc.sync.dma_start(out=sqf_d[:], in_=SQF[:])
            SQB = sg.tile([128, N], f32)
            nc.sync.dma_start(out=SQB[0:NLOC, :],
                              in_=sqf_d[:].to_broadcast([NLOC, N]))
            PRE = sg.tile([NLOC, N], f32)
            nc.vector.tensor_add(PRE[:], A[:], SQB[0:NLOC, :])
            nc.vector.tensor_scalar(out=PRE[:], in0=PRE[:], scalar1=0.0,
                                    scalar2=None, op0=Alu.max)
            DIST = sg.tile([NLOC, N], f32)
            nc.scalar.activation(out=DIST[:], in_=PRE[:], func=Act.Sqrt)

            # masked v' row, bf16
            LBC = sg.tile([128, N], f32)
            nc.sync.dma_start(out=LBC[0:NLOC, :],
                              in_=labrow[:].to_broadcast([NLOC, N]))
            EQB = sg.tile([NLOC, N], f32)
            nc.vector.tensor_scalar(out=EQB[:], in0=LBC[0:NLOC, :], scalar1=LLT[:],
                                    scalar2=BIG, op0=Alu.is_equal, op1=Alu.mult)
            VM = sg.tile([NLOC, N], f32)
            nc.vector.tensor_add(VM[:], DIST[:], EQB[:])
            VMB = sg.tile([NLOC, N], bf16)
            nc.vector.tensor_copy(VMB[:], VM[:])
            vmd = dram.tile([NLOC, N], bf16)
            nc.sync.dma_start(out=vmd[:], in_=VMB[:])

            # dist rows flat in DRAM: the window gather + dp-tile source
            dist_d = dram.tile([1, NLOC * N], f32)
            dfl = dist_d[:]
            dst = bass.AP(tensor=dfl.tensor, offset=0,
                          ap=[[N, NLOC], [1, N]])
            nc.sync.dma_start(out=dst, in_=DIST[:])

            # dp: psum-tile-aligned dist rows (row 32s+2 of tile t holds
            # the dist row of anchor 4t+s); one strided DMA
            DP = sg.tile([128, NT, N], f32)
            sl = DP[2:3, :, :]
            pstr = DP[:].ap[0][0]
            dpdst = bass.AP(tensor=sl.tensor, offset=sl.offset,
                            ap=[[32 * pstr, 4], [N, NT], [1, N]])
            dpsrc = bass.AP(tensor=dfl.tensor, offset=0,
                            ap=[[N, 4], [4 * N, NT], [1, N]])
            nc.gpsimd.dma_start(out=dpdst, in_=dpsrc)

            # window thresholds via indirect gather: TQR[la, p] =
            # dist[la, w_la + p]; offsets woff = 640*la + w_la
            TQR = sg.tile([NLOC, WMAX], f32)
            nc.gpsimd.indirect_dma_start(
                out=TQR[:], out_offset=None,
                in_=dist_d[:],
                in_offset=bass.IndirectOffsetOnAxis(ap=WOFF[:, :1], axis=1),
            )
            tq_ps = ps_tr.tile([WMAX, NLOC], f32, tag="tr", name="tq")
            nc.tensor.transpose(tq_ps[:], TQR[:], ident[0:NLOC, 0:NLOC])
            TQ = sg.tile([WMAX, NLOC], f32)
            nc.vector.tensor_scalar_add(out=TQ[:], in0=tq_ps[:], scalar1=MARGIN)
            nc.vector.tensor_mul(TQ[:], TQ[:], POSW[:])

            # lhsT tiles [128, NLOC, 3] bf16: [t_hi | t_lo | 1]
            LHSB = sg.tile([WMAX, NLOC, 3], bf16)
            nc.vector.tensor_copy(LHSB[:, :, 0], TQ[:])
            thf = sg.tile([WMAX, NLOC], f32)
            nc.vector.tensor_copy(thf[:], LHSB[:, :, 0])
            nc.vector.tensor_sub(thf[:], TQ[:], thf[:])
            nc.vector.tensor_copy(LHSB[:, :, 1], thf[:])
            nc.vector.memset(LHSB[:, :, 2], 1.0)

            # Tsum_a = sum_p t'_ap (for ACT-sign corrections)
            ts_ps = ps_tr.tile([1, NLOC], f32, tag="tr", name="ts_ps")
            nc.tensor.matmul(ts_ps[:], ones[:], TQ[:])
            TSROW = sg.tile([1, NLOC], f32)
            nc.vector.tensor_copy(TSROW[:], ts_ps[:])
            nc.sync.dma_start(out=out2_d[:], in_=TSROW[:])

            # dist row sums (for sign-anchor corrections)
            DSC = sg.tile([NLOC, N], f32)
            DSUM = sg.tile([NLOC, 1], f32)
            nc.scalar.activation(out=DSC[:], in_=DIST[:], func=Act.Identity,
                                 bias=0.0, scale=1.0, accum_out=DSUM[:])

            pro_stack.close()
            wq_stack = ExitStack()
            ps_wq = wq_stack.enter_context(
                tc.tile_pool(name="ps_wq", bufs=3, space="PSUM"))

            DRC = sg.tile([128, NT], f32)
            P2C = sg.tile([128, NT], f32)

            # ---- main loop: one [128, 640] compare + matmul pair/anchor ----
            vb2_cache = {}
            wq = None
            for la in range(NLOC):
                t, s = la // 4, la % 4
                if la % 2 == 0:
                    nanch = min(2, NLOC - la)
                    vb2 = vbp.tile([128, 2, N], bf16, tag="vb", name="vb")
                    sl = vmd[la:la + nanch, :]
                    bsrc = bass.AP(tensor=sl.tensor, offset=sl.offset,
                                   ap=[[0, 128]] + [list(p) for p in sl.ap])
                    nc.gpsimd.dma_start(out=vb2[:, 0:nanch, :], in_=bsrc)
                    vb2_cache[0] = vb2
                vb = vb2_cache[0][:, la % 2, :]
                if s == 0:
                    wq = ps_wq.tile([128, N], f32, tag="wq", name="wq")
                mt = mtp.tile([128, N], bf16, tag="mt", name="mt")
                if _is_act(la):
                    nc.scalar.activation(out=mt[:], in_=vb[:], func=Act.Sign,
                                         bias=TQ[:, la:la + 1], scale=-1.0)
                else:
                    nc.vector.tensor_scalar(out=mt[:], in0=vb[:],
                                            scalar1=TQ[:, la:la + 1],
                                            scalar2=None, op0=Alu.is_lt)
                nc.tensor.matmul(wq[32 * s:32 * s + 3, 0:512],
                                 LHSB[:, la], mt[:, 0:512],
                                 start=True, stop=True,
                                 tile_position=(0, 32 * s))
                nc.tensor.matmul(wq[32 * s:32 * s + 3, 512:N],
                                 LHSB[:, la], mt[:, 512:N],
                                 start=True, stop=True,
                                 tile_position=(0, 32 * s))
                if s == 3:
                    sa = drs.tile([128, N], f32, tag="sa", name="sa")
                    sb = drs.tile([128, N], f32, tag="sb", name="sb")
                    nc.scalar.activation(out=sa[:], in_=wq[:],
                                         func=Act.Identity, bias=0.0, scale=1.0,
                                         accum_out=DRC[:, t:t + 1])
                    nc.vector.scalar_tensor_tensor(out=sb[:], in0=wq[:],
                                                   scalar=1.0,
                                                   in1=DP[:, t, :],
                                                   op0=Alu.mult, op1=Alu.mult,
                                                   accum_out=P2C[:, t:t + 1])

            # ---- stage outputs ----
            OUTS = sg.tile([128, OUTC], f32)
            nc.vector.tensor_copy(OUTS[:, 0:NT], DRC[:])
            nc.vector.tensor_copy(OUTS[:, NT:2 * NT], P2C[:])
            nc.vector.tensor_copy(OUTS[0:NLOC, 2 * NT:2 * NT + 1], DSUM[:])
            nc.gpsimd.dma_start(out=out_d[:], in_=OUTS[:])
            wq_stack.close()

    nc.compile()
    return nc


def _get_program():
    if "nc" not in _CACHE:
        _CACHE["nc"] = _build_program()
    return _CACHE["nc"]


def _make_inputs(embeddings: np.ndarray, labels: np.ndarray):
    e = np.ascontiguousarray(embeddings.reshape(N, D).astype(np.float32))
    lab = labels.reshape(N).astype(np.int64)
    order = np.argsort(lab, kind="stable")
    e_s = e[order]
    lab_s = lab[order].astype(np.float32)
    labi = lab[order]

    # class windows: for sorted position g, w = min(class_start, N-128)
    starts = np.searchsorted(labi, labi, side="left")
    ends = np.searchsorted(labi, labi, side="right")
    assert int((ends - starts).max()) <= WMAX, "class larger than window"
    wof = np.minimum(starts, N - WMAX).astype(np.int64)

    efT = np.ascontiguousarray(e_s.T)                     # [D, N]
    labrow = lab_s.reshape(1, N)

    in_maps = []
    for r in range(NCORES):
        g0 = r * NLOC
        gg = np.arange(g0, g0 + NLOC)
        w = wof[gg]
        woff = (640 * np.arange(NLOC) + w).astype(np.int32).reshape(NLOC, 1)
        jpos = w[None, :] + np.arange(WMAX)[:, None]      # [WMAX, NLOC]
        posw = ((labi[jpos] == labi[gg][None, :])
                & (jpos != gg[None, :])).astype(np.float32)
        in_maps.append({
            "efT": efT,
            "elocT": np.ascontiguousarray(efT[:, g0:g0 + NLOC]),
            "labrow": labrow,
            "llocT": np.ascontiguousarray(lab_s[g0:g0 + NLOC].reshape(NLOC, 1)),
            "woff": woff,
            "posw": posw,
        })
    return in_maps


def run_on_device(embeddings: np.ndarray, labels: np.ndarray, **run_kwargs):
    from concourse.bass_utils import run_bass_kernel_spmd
    nc = _get_program()
    in_maps = _make_inputs(embeddings, labels)
    res = run_bass_kernel_spmd(nc, in_maps, core_ids=list(range(NCORES)),
                               **run_kwargs)
    total = 0.0
    count = 0.0
    for r in range(NCORES):
        o = res.results[r]["out"].astype(np.float64)
        tsum = res.results[r]["out2"].astype(np.float64).reshape(-1)
        dsum = o[0:NLOC, 2 * NT]
        for la in range(NLOC):
            t, s = la // 4, la % 4
            w = o[32 * s + 0, t] + o[32 * s + 1, t]
            q = o[32 * s + 2, t]
            p2 = o[32 * s + 2, NT + t]
            if _is_act(la):   # sign anchor: M = (M' + 1)/2
                w = 0.5 * w + 0.5 * N * tsum[la]
                q = 0.5 * q + 0.5 * WMAX * N
                p2 = 0.5 * p2 + 0.5 * WMAX * dsum[la]
            total += w - p2
            count += q
    return total, count, res


def kernel(embeddings: np.ndarray, labels: np.ndarray):
    embeddings = np.asarray(embeddings)
    labels = np.asarray(labels)
    total, count, _ = run_on_device(embeddings, labels)

    lab = np.asarray(labels).reshape(-1)
    cnt = np.bincount(lab.astype(np.int64), minlength=1)
    per = cnt[lab.astype(np.int64)]
    num_valid = int(((per - 1) * (N - per)).sum())

    nv = np.float32(num_valid)
    ne = np.float32(count)
    tot = np.float32(total)
    if ne > 0:
        loss = np.float32(tot / np.maximum(ne, np.float32(1.0)))
    else:
        loss = np.float32(0.0)
    frac = np.float32(ne / (nv + np.float32(1e-16)))
    return (np.array(loss, np.float32), np.array(nv, np.float32),
            np.array(ne, np.float32), np.array(frac, np.float32))
